# revision 1
# baseline (speedup 1.0000x reference)
"""Trainium2 Bass kernel v2 for GroundwaterModel Jacobi pseudo-timestepping.

Layout (per core, x sharded 128 rows/core):
  partition p (0..127), block b (0..7)  <->  y = 8p + b   (y interleaved)
  in-block free col m (0..F-1)          <->  x = 128c - (H+1) + m
  F = 130 + 2H:  [pad | H left ghosts | 128 owned | H right ghosts | pad]
  One SBUF tile Q[128, 8F] fp16 holds the full per-core state.

Update p' = A.p_xp + B.p_xm + C.p_yp + D.p_ym + E as five PSUM
accumulations of *unshifted* fp16 streams (source-aligned coefficients,
host-precomputed): shifts happen in the PE matmul read offsets:
  x+-1  -> flat free offset +-1 (pads isolate blocks)
  y+-1  -> free offset +-F (b+-1) plus a tiny partition-shift wrap matmul
           for the b=7->0 / b=0->7 cases.
DVE does only 4 fp16 muls/step (2x mode); ACT drains PSUM->fp16.

Halo: ghosts H=17 wide refresh every 17 steps (5 refreshes) via two
pairwise AllGathers (rounds [[0,1],[2,3],..] and [[0,7],[1,2],..]) and
data-driven mask selection; grid-edge cores use mirrored ghosts
(reflection principle == reference's edge replication).
"""

import numpy as np

GRID = 1024
NCORES = 8
P = 128
H = 17
F = 130 + 2 * H          # 164
FF = 8 * F               # 1312
NB = 8
OWN0 = H + 1             # first owned in-block col (18)
TS = 100

_cached = {}


def _host_inputs(u, f, n_cores, time_steps):
    N = u.shape[0]
    h = 1.0 / (N - 1)
    u = u.astype(np.float64)
    f = f.astype(np.float64)
    eu = np.exp(u)
    eu_xm = np.concatenate([eu[:1, :], eu[:-1, :]], 0)
    eu_ym = np.concatenate([eu[:, :1], eu[:, :-1]], 1)
    den = 2.0 * eu + eu_xm + eu_ym
    A = eu / den
    B = eu_xm / den
    C = eu / den
    D = eu_ym / den
    E = (h * h) * f / den
    for arr in (A, B, C, D):
        arr[:, 0] = 0.0
        arr[:, -1] = 0.0
    xs = np.arange(N, dtype=np.float64) * h
    E[:, 0] = xs
    E[:, -1] = 1.0 - xs

    def til(arr):  # [F, N] fp -> [128, 8F] fp16 tile layout
        return np.ascontiguousarray(
            arr.reshape(F, P, NB).transpose(1, 2, 0).reshape(P, NB * F)
        ).astype(np.float16)

    in_maps = []
    for c in range(n_cores):
        r0 = c * P
        At = np.zeros((F, N)); Bt = np.zeros((F, N))
        Ct = np.zeros((F, N)); Dt = np.zeros((F, N)); Et = np.zeros((F, N))
        for m in range(1, F - 1):
            x = r0 - (H + 1) + m
            if 0 <= x < N:
                At[m], Bt[m], Ct[m], Dt[m], Et[m] = A[x], B[x], C[x], D[x], E[x]
            else:
                xt = -1 - x if x < 0 else 2 * N - 1 - x
                # mirror: x-direction roles swap
                At[m], Bt[m], Ct[m], Dt[m], Et[m] = B[xt], A[xt], C[xt], D[xt], E[xt]
        # source-aligned (storage-space) shifts
        Ash = np.zeros_like(At); Ash[1:] = At[:-1]
        Bsh = np.zeros_like(Bt); Bsh[:-1] = Bt[1:]
        Csh = np.zeros_like(Ct); Csh[:, 1:] = Ct[:, :-1]
        Dsh = np.zeros_like(Dt); Dsh[:, :-1] = Dt[:, 1:]

        ident16 = np.eye(P, dtype=np.float16)
        supT = np.zeros((P, P), dtype=np.float16)
        for p in range(P - 1):
            supT[p + 1, p] = 1.0    # out[p] = in[p+1]
        sdnT = np.zeros((P, P), dtype=np.float16)
        for p in range(1, P):
            sdnT[p - 1, p] = 1.0    # out[p] = in[p-1]
        ident32 = np.eye(P, dtype=np.float32)

        def mk_mask(val):
            return np.full((P, NB, H), val, dtype=np.uint8)

        # world-AG selection masks: left ghosts come from slot c-1's right
        # section (mirror of own for core 0); right from slot c+1's left
        # section (mirror for core 7).
        im = {
            "ash": til(Ash), "bsh": til(Bsh), "csh": til(Csh),
            "dsh": til(Dsh), "e0": til(Et),
            "ident16": ident16, "supT": supT, "sdnT": sdnT, "ident32": ident32,
        }
        for s in range(1, n_cores - 1):
            im[f"mL{s}"] = mk_mask(1.0 if c == s + 1 else 0.0)
            im[f"mR{s}"] = mk_mask(1.0 if c == s - 1 else 0.0)
        im["mR7"] = mk_mask(1.0 if c == n_cores - 2 else 0.0)
        im["mLM"] = mk_mask(1.0 if c == 0 else 0.0)
        im["mRM"] = mk_mask(1.0 if c == n_cores - 1 else 0.0)
        in_maps.append(im)
    return in_maps


def _build(n_cores, time_steps, repeats=1):
    import concourse.bass as bass
    import concourse.bacc as bacc
    import concourse.mybir as mybir
    from concourse.tile import TileContext

    f16 = mybir.dt.float16
    f32 = mybir.dt.float32
    AF = mybir.ActivationFunctionType
    OP = mybir.AluOpType

    nc = bacc.Bacc("TRN2", target_bir_lowering=False, debug=False,
                   num_devices=n_cores)
    dp = nc.declare_dram_parameter
    ash_d = dp("ash", [P, FF], f16, isOutput=False)
    bsh_d = dp("bsh", [P, FF], f16, isOutput=False)
    csh_d = dp("csh", [P, FF], f16, isOutput=False)
    dsh_d = dp("dsh", [P, FF], f16, isOutput=False)
    e0_d = dp("e0", [P, FF], f16, isOutput=False)
    i16_d = dp("ident16", [P, P], f16, isOutput=False)
    sup_d = dp("supT", [P, P], f16, isOutput=False)
    sdn_d = dp("sdnT", [P, P], f16, isOutput=False)
    i32_d = dp("ident32", [P, P], f32, isOutput=False)
    mask_names = ([f"mL{s}" for s in range(1, n_cores - 1)]
                  + [f"mR{s}" for s in range(1, n_cores)]
                  + ["mLM", "mRM"])
    u8 = mybir.dt.uint8
    mask_d = {nm: dp(nm, [P, NB, H], u8, isOutput=False)
              for nm in mask_names}
    pout_d = dp("pout", [P, NB, P], f32, isOutput=True)

    rg = [list(range(n_cores))]

    refresh_steps = set()
    t = 1 + H
    while t < time_steps:
        refresh_steps.add(t)
        t += H

    # Per-PSUM-bank chunking: ops issued at bank granularity so the Tile
    # scheduler can overlap DVE muls / PE matmuls / ACT drains across
    # banks and adjacent steps.
    F7 = 7 * F
    banks = [(0, 512), (512, 1024), (1024, FF)]

    with TileContext(nc) as tc:
        with (
            tc.tile_pool(name="coef", bufs=1) as coef,
            tc.tile_pool(name="work", bufs=2) as work,
            tc.tile_pool(name="qp", bufs=2, space="PSUM") as qp,
            tc.tile_pool(name="tp", bufs=2, space="PSUM") as tp,
            tc.tile_pool(name="dramp", bufs=2, space="DRAM") as dramp,
        ):
            ash = coef.tile([P, FF], f16, name="ash_t")
            bsh = coef.tile([P, FF], f16, name="bsh_t")
            csh = coef.tile([P, FF], f16, name="csh_t")
            dsh = coef.tile([P, FF], f16, name="dsh_t")
            e0 = coef.tile([P, FF], f16, name="e0_t")
            i16 = coef.tile([P, P], f16, name="i16_t")
            supT = coef.tile([P, P], f16, name="sup_t")
            sdnT = coef.tile([P, P], f16, name="sdn_t")
            i32 = coef.tile([P, P], f32, name="i32_t")
            masks = {nm: coef.tile([P, NB, H], mybir.dt.uint8,
                                   name=f"{nm}_t")
                     for nm in mask_d}
            outsb = coef.tile([P, NB * P], f32, name="outsb")
            for sb_t, d_t in [(ash, ash_d), (bsh, bsh_d), (csh, csh_d),
                              (dsh, dsh_d), (e0, e0_d), (i16, i16_d),
                              (supT, sup_d), (sdnT, sdn_d), (i32, i32_d)]:
                nc.sync.dma_start(out=sb_t[:, :], in_=d_t[:, :])
            for nm in mask_d:
                nc.sync.dma_start(out=masks[nm][:, :, :], in_=mask_d[nm][:, :, :])

            V = nc.vector
            mm = nc.tensor.matmul

            def bank_matmuls(ps, sxp, sxm, syp, sym, k):
                # No E matmul: E is folded into the DVE drain (q = ps+e0).
                # The bank-clearing start=True matmul must cover the full
                # bank: yp spans banks 0/1 fully, ym spans bank 2 fully.
                lo, hi = banks[k]
                if k < 2:
                    # banks 0/1 carry E in PSUM (drained by ACT as a plain
                    # copy); E is also the dep-free bank-clearing matmul.
                    mm(ps[:, lo:hi], i16[:, :], e0[:, lo:hi],
                       start=True, stop=False)
                    mm(ps[:, lo:hi], i16[:, :], syp[:, lo + F:hi + F],
                       start=False, stop=False)
                    mm(ps[:, max(lo, F):hi], i16[:, :],
                       sym[:, max(lo, F) - F:hi - F],
                       start=False, stop=False)
                else:
                    mm(ps[:, lo:hi], i16[:, :], sym[:, lo - F:hi - F],
                       start=True, stop=False)
                    mm(ps[:, lo:F7], i16[:, :], syp[:, lo + F:F7 + F],
                       start=False, stop=False)
                h2 = min(hi, FF - 1)
                mm(ps[:, lo:h2], i16[:, :], sxp[:, lo + 1:h2 + 1],
                   start=False, stop=False)
                l2 = max(lo, 1)
                # bank1's region is finished here; banks 0/2 finish at the
                # wraps issued after all bank groups.
                mm(ps[:, l2:hi], i16[:, :], sxm[:, l2 - 1:hi - 1],
                   start=False, stop=(k == 1))

            def wrap_matmuls(ps, syp, sym):
                # wraps read the b=7/b=0 stream chunks (last DVE chunks);
                # issued after all identity matmuls so the in-order PE
                # queue never stalls mid-sequence waiting on DVE.
                mm(ps[:, 0:F], sdnT[:, :], sym[:, F7:FF],
                   start=False, stop=True)
                mm(ps[:, F7:FF], supT[:, :], syp[:, 0:F],
                   start=False, stop=True)

            def refresh(q, t):
                Qr = q[:, :].rearrange("p (b f) -> p b f", b=NB)
                contrib = work.tile([P, 2, NB, H], f16, tag="contrib",
                                    name=f"contrib_{t}")
                V.tensor_copy(contrib[:, 0, :, :], Qr[:, :, OWN0:OWN0 + H])
                V.tensor_copy(contrib[:, 1, :, :],
                              Qr[:, :, OWN0 + P - H:OWN0 + P])
                bounce = dramp.tile([P, 2, NB, H], f16, tag="bounce",
                                    name=f"bounce_{t}")
                nc.sync.dma_start(out=bounce[:, :, :, :],
                                  in_=contrib[:, :, :, :])
                gath = dramp.tile([n_cores * P, 2, NB, H], f16, tag="gath",
                                  addr_space="Shared", name=f"gath_{t}")
                nc.gpsimd.collective_compute(
                    "AllGather", mybir.AluOpType.bypass,
                    ins=[bounce.opt()], outs=[gath.opt()],
                    replica_groups=rg)
                gsb = work.tile([P, n_cores, 2, NB, H], f16, tag="gsb",
                                name=f"gsb_{t}")
                for s in range(n_cores):
                    nc.sync.dma_start(out=gsb[:, s, :, :, :],
                                      in_=gath[s * P:(s + 1) * P, :, :, :])
                # left ghosts <- slot c-1 right-section; core 0 mirrors own
                gl = work.tile([P, NB, H], f16, tag="gl", name=f"gl_{t}")
                V.tensor_copy(gl[:, :, :], gsb[:, 0, 1, :, :])
                for s in range(1, n_cores - 1):
                    V.copy_predicated(gl[:, :, :], masks[f"mL{s}"][:, :, :],
                                      gsb[:, s, 1, :, :])
                V.copy_predicated(gl[:, :, :], masks["mLM"][:, :, :],
                                  Qr[:, :, OWN0 + H - 1:OWN0 - 1:-1])
                V.tensor_copy(Qr[:, :, 1:1 + H], gl[:, :, :])
                # right ghosts <- slot c+1 left-section; core 7 mirrors own
                gr = work.tile([P, NB, H], f16, tag="gr", name=f"gr_{t}")
                V.tensor_copy(gr[:, :, :], gsb[:, 1, 0, :, :])
                for s in range(2, n_cores):
                    V.copy_predicated(gr[:, :, :], masks[f"mR{s}"][:, :, :],
                                      gsb[:, s, 0, :, :])
                V.copy_predicated(gr[:, :, :], masks["mRM"][:, :, :],
                                  Qr[:, :, OWN0 + P - 1:OWN0 + P - H - 1:-1])
                V.tensor_copy(Qr[:, :, OWN0 + P:OWN0 + P + H], gr[:, :, :])

            for rep in range(repeats):
                q = work.tile([P, FF], f16, tag="q", name=f"q1_r{rep}")
                V.tensor_copy(q[:, :], e0[:, :])
                pf = None
                for t in range(2, time_steps + 1):
                    sxp = work.tile([P, FF], f16, tag="sxp", name=f"sxp_{rep}_{t}")
                    sxm = work.tile([P, FF], f16, tag="sxm", name=f"sxm_{rep}_{t}")
                    syp = work.tile([P, FF], f16, tag="syp", name=f"syp_{rep}_{t}")
                    sym = work.tile([P, FF], f16, tag="sym", name=f"sym_{rep}_{t}")
                    # bank-2 chunks first: DVE's own drain wrote q bank 2,
                    # so these muls run while ACT still drains banks 0/1.
                    for st, cf in ((syp, csh), (sym, dsh), (sxp, ash),
                                   (sxm, bsh)):
                        for lo, hi in (banks[2], banks[0], banks[1]):
                            V.tensor_mul(st[:, lo:hi], cf[:, lo:hi],
                                         q[:, lo:hi])
                    ps = qp.tile([P, FF], f32, tag="ps", name=f"ps_{rep}_{t}")
                    for k in range(3):
                        bank_matmuls(ps, sxp, sxm, syp, sym, k)
                    wrap_matmuls(ps, syp, sym)
                    # Split drain after ALL matmuls: ACT copies banks 0/1
                    # (E already accumulated there), DVE adds e0 to bank 2.
                    # Disjoint PSUM banks; ACT||DVE reads are documented
                    # safe and neither overlaps PE.
                    b2 = banks[2][0]
                    if t < time_steps:
                        q = work.tile([P, FF], f16, tag="q", name=f"q_{rep}_{t}")
                        V.tensor_add(q[:, b2:FF], ps[:, b2:FF], e0[:, b2:FF])
                        nc.scalar.activation(q[:, 0:b2], ps[:, 0:b2], AF.Copy)
                        if t in refresh_steps:
                            refresh(q, f"{rep}_{t}")
                    else:
                        pf = work.tile([P, FF], f32, tag="pf", name=f"pf_{rep}")
                        V.tensor_add(pf[:, b2:FF], ps[:, b2:FF], e0[:, b2:FF])
                        nc.scalar.activation(pf[:, 0:b2], ps[:, 0:b2], AF.Copy)

            # final transpose to [x, y-blockmajor] and write out
            for b in range(NB):
                # full-bank tile: transpose's start=True clears the whole
                # physical bank, so no two pst buffers may share one.
                pst = tp.tile([P, 512], f32, tag="pst", name=f"pst_{b}")
                nc.tensor.transpose(pst[:, 0:P],
                                    pf[:, b * F + OWN0:b * F + OWN0 + P],
                                    i32[:, :])
                nc.scalar.activation(outsb[:, b * P:(b + 1) * P],
                                     pst[:, 0:P], AF.Copy)
            for b in range(NB):
                nc.sync.dma_start(out=pout_d[:, b, :],
                                  in_=outsb[:, b * P:(b + 1) * P])

    nc.finalize()
    return nc


def _get_nc(n_cores, time_steps, repeats=1):
    key = (n_cores, time_steps, repeats)
    if key not in _cached:
        _cached[key] = _build(n_cores, time_steps, repeats)
    return _cached[key]


def kernel(u, f, time_steps):
    from concourse.bass_utils import run_bass_kernel_spmd

    u = np.asarray(u)
    f = np.asarray(f)
    ts = int(time_steps)
    N = u.shape[0]
    nc = _get_nc(NCORES, ts)
    in_maps = _host_inputs(u, f, NCORES, ts)
    res = run_bass_kernel_spmd(nc, in_maps, list(range(NCORES))).results
    out = np.empty((N, N), dtype=np.float32)
    for c in range(NCORES):
        po = res[c]["pout"]  # [128, 8, 128] -> [128 x, 1024 y]
        out[c * P:(c + 1) * P] = po.transpose(0, 2, 1).reshape(P, N)
    return out



# revision 17
# speedup vs baseline: 16.5112x; 16.5112x over previous
"""Trainium2 Bass kernel v3 for GroundwaterModel Jacobi pseudo-timestepping.

Layout (per core, x sharded 128 rows/core):
  partition p (0..127), block b (0..7)  <->  y = 8p + b   (y interleaved)
  in-block free col m (0..F-1)          <->  x = 128c - (H+1) + m
  F = 130 + 2H:  [pad | H left ghosts | 128 owned | H right ghosts | pad]
  One SBUF tile q[128, F*8] fp16 holds the full per-core state.

Update p' = A.p_xp + B.p_xm + C.p_yp + D.p_ym + E:
  - DVE computes all 4 coefficient streams in one broadcast mul per chunk:
    s[:, k, r] = cc[:, k, r] * q[:, r]  (cc = [csh,dsh,ash,bsh] stacked)
  - PE accumulates E + the 4 shifted streams into PSUM via identity /
    partition-shift matmuls (shifts = read offsets; y wrap via supT/sdnT)
  - drains: ACT copies PSUM banks 0/1 -> q fp16, Pool copies bank 2.
  Chunk boundaries are chosen so each drain immediately unblocks the next
  step's corresponding mul chunk (cross-step pipelining):
    DVE order: A[0,512) W[1148,1312) B1[512,676) B2[676,1024) C[1024,1148)
    A needs drain_A(t-1), W/C need drain_C(t-1), B1/B2 need drain_B(t-1).

Halo: ghosts H=17 wide refresh every 17 steps (5 refreshes) via two
AllGathers-free scheme: one AllGather of own margins and data-driven mask
selection; grid-edge cores use mirrored ghosts (reflection principle ==
reference's edge replication).  Measured ~free on HW.
"""

import numpy as np

GRID = 1024
NCORES = 8
P = 128
H = 17
F = 130 + 2 * H          # 164
FF = 8 * F               # 1312
NB = 8
OWN0 = H + 1             # first owned in-block col (18)
TS = 100

_cached = {}
DISABLE_REFRESH = False


def _host_inputs(u, f, n_cores, time_steps):
    N = u.shape[0]
    h = 1.0 / (N - 1)
    u = u.astype(np.float64)
    f = f.astype(np.float64)
    eu = np.exp(u)
    eu_xm = np.concatenate([eu[:1, :], eu[:-1, :]], 0)
    eu_ym = np.concatenate([eu[:, :1], eu[:, :-1]], 1)
    den = 2.0 * eu + eu_xm + eu_ym
    A = eu / den
    B = eu_xm / den
    C = eu / den
    D = eu_ym / den
    E = (h * h) * f / den
    for arr in (A, B, C, D):
        arr[:, 0] = 0.0
        arr[:, -1] = 0.0
    xs = np.arange(N, dtype=np.float64) * h
    E[:, 0] = xs
    E[:, -1] = 1.0 - xs

    def til(arr):  # [F, N] fp -> [128, 8F] fp16 tile layout
        return np.ascontiguousarray(
            arr.reshape(F, P, NB).transpose(1, 2, 0).reshape(P, NB * F)
        ).astype(np.float16)

    in_maps = []
    for c in range(n_cores):
        r0 = c * P
        At = np.zeros((F, N)); Bt = np.zeros((F, N))
        Ct = np.zeros((F, N)); Dt = np.zeros((F, N)); Et = np.zeros((F, N))
        for m in range(1, F - 1):
            x = r0 - (H + 1) + m
            if 0 <= x < N:
                At[m], Bt[m], Ct[m], Dt[m], Et[m] = A[x], B[x], C[x], D[x], E[x]
            else:
                xt = -1 - x if x < 0 else 2 * N - 1 - x
                # mirror: x-direction roles swap
                At[m], Bt[m], Ct[m], Dt[m], Et[m] = B[xt], A[xt], C[xt], D[xt], E[xt]
        # source-aligned (storage-space) shifts
        Ash = np.zeros_like(At); Ash[1:] = At[:-1]
        Bsh = np.zeros_like(Bt); Bsh[:-1] = Bt[1:]
        Csh = np.zeros_like(Ct); Csh[:, 1:] = Ct[:, :-1]
        Dsh = np.zeros_like(Dt); Dsh[:, :-1] = Dt[:, 1:]

        # stream order k: 0=sym(dsh), 1=syp(csh), 2=sxp(ash), 3=sxm(bsh)
        cc = np.stack([til(Dsh), til(Csh), til(Ash), til(Bsh)], axis=1)

        ident16 = np.eye(P, dtype=np.float16)
        supT = np.zeros((P, P), dtype=np.float16)
        for p in range(P - 1):
            supT[p + 1, p] = 1.0    # out[p] = in[p+1]
        sdnT = np.zeros((P, P), dtype=np.float16)
        for p in range(1, P):
            sdnT[p - 1, p] = 1.0    # out[p] = in[p-1]
        ident32 = np.eye(P, dtype=np.float32)

        def mk_mask(val):
            return np.full((P, NB, H), val, dtype=np.uint8)

        # world-AG selection masks: left ghosts come from slot c-1's right
        # section (mirror of own for core 0); right from slot c+1's left
        # section (mirror for core 7).
        im = {
            "cc": np.ascontiguousarray(cc), "e0": til(Et),
            "ident16": ident16, "supT": supT, "sdnT": sdnT, "ident32": ident32,
        }
        for s in range(1, n_cores - 1):
            im[f"mL{s}"] = mk_mask(1.0 if c == s + 1 else 0.0)
            im[f"mR{s}"] = mk_mask(1.0 if c == s - 1 else 0.0)
        im["mR7"] = mk_mask(1.0 if c == n_cores - 2 else 0.0)
        im["mLM"] = mk_mask(1.0 if c == 0 else 0.0)
        im["mRM"] = mk_mask(1.0 if c == n_cores - 1 else 0.0)
        in_maps.append(im)
    return in_maps


def _build(n_cores, time_steps, repeats=1, single_core_profile=False):
    import concourse.bass as bass
    import concourse.bacc as bacc
    import concourse.mybir as mybir
    from concourse.tile import TileContext

    f16 = mybir.dt.float16
    f32 = mybir.dt.float32
    AF = mybir.ActivationFunctionType
    OP = mybir.AluOpType

    nc = bacc.Bacc("TRN2", target_bir_lowering=False, debug=False,
                   num_devices=(1 if single_core_profile else n_cores))
    dp = nc.declare_dram_parameter
    cc_d = dp("cc", [P, 4, FF], f16, isOutput=False)
    e0_d = dp("e0", [P, FF], f16, isOutput=False)
    i16_d = dp("ident16", [P, P], f16, isOutput=False)
    sup_d = dp("supT", [P, P], f16, isOutput=False)
    sdn_d = dp("sdnT", [P, P], f16, isOutput=False)
    i32_d = dp("ident32", [P, P], f32, isOutput=False)
    mask_names = ([f"mL{s}" for s in range(1, n_cores - 1)]
                  + [f"mR{s}" for s in range(1, n_cores)]
                  + ["mLM", "mRM"])
    u8 = mybir.dt.uint8
    mask_d = {nm: dp(nm, [P, NB, H], u8, isOutput=False)
              for nm in mask_names}
    pout_d = dp("pout", [P, NB, P], f32, isOutput=True)

    rg = [list(range(n_cores))]

    refresh_steps = set()
    t = 1 + H
    while t < time_steps:
        refresh_steps.add(t)
        t += H
    if single_core_profile or DISABLE_REFRESH:
        refresh_steps = set()

    F7 = 7 * F

    with TileContext(nc) as tc:
        with (
            tc.tile_pool(name="coef", bufs=1) as coef,
            tc.tile_pool(name="work", bufs=2) as work,
            tc.tile_pool(name="qp", bufs=2, space="PSUM") as qp,
            tc.tile_pool(name="tp", bufs=2, space="PSUM") as tp,
            tc.tile_pool(name="dramp", bufs=2, space="DRAM") as dramp,
        ):
            cc = coef.tile([P, 4, FF], f16, name="cc_t")
            e0 = coef.tile([P, FF], f16, name="e0_t")
            i16 = coef.tile([P, P], f16, name="i16_t")
            supT = coef.tile([P, P], f16, name="sup_t")
            sdnT = coef.tile([P, P], f16, name="sdn_t")
            i32 = coef.tile([P, P], f32, name="i32_t")
            masks = {nm: coef.tile([P, NB, H], mybir.dt.uint8,
                                   name=f"{nm}_t")
                     for nm in mask_d}
            outsb = coef.tile([P, NB * P], f32, name="outsb")
            nc.sync.dma_start(out=cc[:, :, :], in_=cc_d[:, :, :])
            for sb_t, d_t in [(e0, e0_d), (i16, i16_d),
                              (supT, sup_d), (sdnT, sdn_d), (i32, i32_d)]:
                nc.sync.dma_start(out=sb_t[:, :], in_=d_t[:, :])
            for nm in mask_d:
                nc.sync.dma_start(out=masks[nm][:, :, :], in_=mask_d[nm][:, :, :])

            V = nc.vector
            G = nc.gpsimd
            mm = nc.tensor.matmul

            def mul_chunk(s, q, lo, hi, k0=0, k1=4, eng=None):
                nk = k1 - k0
                qb = q[:, lo:hi].rearrange("p (k m) -> p k m", k=1)
                qb = qb.broadcast_to([P, nk, hi - lo])
                (eng or V).tensor_mul(s[:, k0:k1, lo:hi], cc[:, k0:k1, lo:hi],
                                      qb)

            def refresh(q, t):
                Qr = q[:, :].rearrange("p (b f) -> p b f", b=NB)
                contrib = work.tile([P, 2, NB, H], f16, tag="contrib",
                                    name=f"contrib_{t}")
                V.tensor_copy(contrib[:, 0, :, :], Qr[:, :, OWN0:OWN0 + H])
                V.tensor_copy(contrib[:, 1, :, :],
                              Qr[:, :, OWN0 + P - H:OWN0 + P])
                bounce = dramp.tile([P, 2, NB, H], f16, tag="bounce",
                                    name=f"bounce_{t}")
                nc.sync.dma_start(out=bounce[:, :, :, :],
                                  in_=contrib[:, :, :, :])
                gath = dramp.tile([n_cores * P, 2, NB, H], f16, tag="gath",
                                  addr_space="Shared", name=f"gath_{t}")
                nc.gpsimd.collective_compute(
                    "AllGather", mybir.AluOpType.bypass,
                    ins=[bounce.opt()], outs=[gath.opt()],
                    replica_groups=rg)
                gsb = work.tile([P, n_cores, 2, NB, H], f16, tag="gsb",
                                name=f"gsb_{t}")
                for s in range(n_cores):
                    nc.sync.dma_start(out=gsb[:, s, :, :, :],
                                      in_=gath[s * P:(s + 1) * P, :, :, :])
                # left ghosts <- slot c-1 right-section; core 0 mirrors own
                gl = work.tile([P, NB, H], f16, tag="gl", name=f"gl_{t}")
                V.tensor_copy(gl[:, :, :], gsb[:, 0, 1, :, :])
                for s in range(1, n_cores - 1):
                    V.copy_predicated(gl[:, :, :], masks[f"mL{s}"][:, :, :],
                                      gsb[:, s, 1, :, :])
                V.copy_predicated(gl[:, :, :], masks["mLM"][:, :, :],
                                  Qr[:, :, OWN0 + H - 1:OWN0 - 1:-1])
                V.tensor_copy(Qr[:, :, 1:1 + H], gl[:, :, :])
                # right ghosts <- slot c+1 left-section; core 7 mirrors own
                gr = work.tile([P, NB, H], f16, tag="gr", name=f"gr_{t}")
                V.tensor_copy(gr[:, :, :], gsb[:, 1, 0, :, :])
                for s in range(2, n_cores):
                    V.copy_predicated(gr[:, :, :], masks[f"mR{s}"][:, :, :],
                                      gsb[:, s, 0, :, :])
                V.copy_predicated(gr[:, :, :], masks["mRM"][:, :, :],
                                  Qr[:, :, OWN0 + P - 1:OWN0 + P - H - 1:-1])
                V.tensor_copy(Qr[:, :, OWN0 + P:OWN0 + P + H], gr[:, :, :])

            for rep in range(repeats):
                q = work.tile([P, FF], f16, tag="q", name=f"q1_r{rep}")
                V.tensor_copy(q[:, :], e0[:, :])
                pf = None
                for t in range(2, time_steps + 1):
                    s = work.tile([P, 4, FF], f16, tag="s",
                                  name=f"s_{rep}_{t}")
                    # DVE chunk order; each chunk's q-range is unblocked by
                    # one drain of the previous step:
                    #   A    <- drain_A   symw/W/C <- drain_C   B1/B2 <- drain_B
                    mul_chunk(s, q, 0, 512)                  # A      (DVE)
                    mul_chunk(s, q, F7, FF, 0, 1, eng=G)     # symw   (Pool)
                    mul_chunk(s, q, 1024, F7, eng=G)         # C      (Pool)
                    mul_chunk(s, q, 512, 676)                # B1     (DVE)
                    mul_chunk(s, q, 676, 1024)               # B2     (DVE)
                    mul_chunk(s, q, F7, FF, 1, 4)            # W      (DVE, last)
                    # one PSUM tile per bank: drains then depend only on
                    # their own bank's accumulation chain
                    psA = qp.tile([P, 512], f32, tag="psA",
                                  name=f"psA_{rep}_{t}")
                    psB = qp.tile([P, 512], f32, tag="psB",
                                  name=f"psB_{rep}_{t}")
                    psC = qp.tile([P, 288], f32, tag="psC",
                                  name=f"psC_{rep}_{t}")
                    sym = s[:, 0, :]; syp = s[:, 1, :]
                    sxp = s[:, 2, :]; sxm = s[:, 3, :]
                    # E inits (dep-free bank clears)
                    mm(psA[:, 0:512], i16[:, :], e0[:, 0:512],
                       start=True, stop=False)
                    mm(psB[:, 0:512], i16[:, :], e0[:, 512:1024],
                       start=True, stop=False)
                    mm(psC[:, 0:288], i16[:, :], e0[:, 1024:FF],
                       start=True, stop=False)
                    # bank A group: A-fed parts first, B1-fed tail -> the
                    # drain_A chain starts right after chunk B1's 1st cols
                    mm(psA[:, F:512], i16[:, :], sym[:, 0:512 - F],
                       start=False, stop=False)                     # ym_A
                    mm(psA[:, 1:512], i16[:, :], sxm[:, 0:511],
                       start=False, stop=False)                     # xm_A
                    mm(psA[:, 0:348], i16[:, :], syp[:, F:512],
                       start=False, stop=False)                     # yp_Aa
                    mm(psA[:, 0:511], i16[:, :], sxp[:, 1:512],
                       start=False, stop=False)                     # xp_Aa
                    mm(psA[:, 0:F], sdnT[:, :], sym[:, F7:FF],
                       start=False, stop=False)                     # wrap1
                    mm(psA[:, 511:512], i16[:, :], sxp[:, 512:513],
                       start=False, stop=False)                     # xp_Ab
                    mm(psA[:, 348:512], i16[:, :], syp[:, 512:676],
                       start=False, stop=True)                      # yp_Ab
                    # drain targets
                    if t < time_steps:
                        qn = work.tile([P, FF], f16, tag="q",
                                       name=f"q_{rep}_{t}")
                    else:
                        qn = work.tile([P, FF], f32, tag="pf",
                                       name=f"pf_{rep}")
                        pf = qn
                    ACT = nc.scalar.activation
                    # bank B parts fed by B1
                    mm(psB[:, 0:328], i16[:, :], sym[:, 348:676],
                       start=False, stop=False)                     # ym_B1
                    mm(psB[:, 0:165], i16[:, :], sxm[:, 511:676],
                       start=False, stop=False)                     # xm_B1
                    mm(psB[:, 0:163], i16[:, :], sxp[:, 513:676],
                       start=False, stop=False)                     # xp_B1
                    # drain_A right after its stop (yp_Ab above)
                    ACT(qn[:, 0:512], psA[:, 0:512], AF.Copy)
                    # bank B parts fed by B2
                    mm(psB[:, 328:512], i16[:, :], sym[:, 676:860],
                       start=False, stop=False)                     # ym_B2
                    mm(psB[:, 165:512], i16[:, :], sxm[:, 676:1023],
                       start=False, stop=False)                     # xm_B2
                    mm(psB[:, 163:511], i16[:, :], sxp[:, 676:1024],
                       start=False, stop=False)                     # xp_B2
                    mm(psB[:, 0:348], i16[:, :], syp[:, 676:1024],
                       start=False, stop=False)                     # yp_Ba
                    mm(psC[:, 0:164], i16[:, :], sym[:, 860:1024],
                       start=False, stop=False)                     # ym_C1
                    # fed by C (Pool)
                    mm(psB[:, 511:512], i16[:, :], sxp[:, 1024:1025],
                       start=False, stop=False)                     # xp_B3
                    mm(psB[:, 348:472], i16[:, :], syp[:, 1024:1148],
                       start=False, stop=False)                     # yp_Bb1
                    mm(psC[:, 164:288], i16[:, :], sym[:, 1024:1148],
                       start=False, stop=False)                     # ym_C2
                    # fed by W: finish bank B asap -> early drain_B
                    mm(psC[:, 0:124], i16[:, :], syp[:, 1024 + F:FF],
                       start=False, stop=False)                     # yp_C
                    mm(psB[:, 472:512], i16[:, :], syp[:, 1148:1188],
                       start=False, stop=True)                      # yp_Bb2
                    ACT(qn[:, 512:1024], psB[:, 0:512], AF.Copy)    # drain_B
                    mm(psC[:, 0:288], i16[:, :], sxm[:, 1023:1311],
                       start=False, stop=False)                     # xm_C
                    mm(psC[:, 0:287], i16[:, :], sxp[:, 1025:FF],
                       start=False, stop=False)                     # xp_C
                    mm(psC[:, 124:288], supT[:, :], syp[:, 0:F],
                       start=False, stop=True)                      # wrap2
                    ACT(qn[:, 1024:FF], psC[:, 0:288], AF.Copy)     # drain_C
                    if t < time_steps:
                        q = qn
                        if t in refresh_steps:
                            refresh(q, f"{rep}_{t}")

            # final transpose to [x, y-blockmajor] and write out
            for b in range(NB):
                # full-bank tile: transpose's start=True clears the whole
                # physical bank, so no two pst buffers may share one.
                pst = tp.tile([P, 512], f32, tag="pst", name=f"pst_{b}")
                nc.tensor.transpose(pst[:, 0:P],
                                    pf[:, b * F + OWN0:b * F + OWN0 + P],
                                    i32[:, :])
                nc.scalar.activation(outsb[:, b * P:(b + 1) * P],
                                     pst[:, 0:P], AF.Copy)
            for b in range(NB):
                nc.sync.dma_start(out=pout_d[:, b, :],
                                  in_=outsb[:, b * P:(b + 1) * P])

    nc.finalize()
    return nc


def _get_nc(n_cores, time_steps, repeats=1):
    key = (n_cores, time_steps, repeats)
    if key not in _cached:
        _cached[key] = _build(n_cores, time_steps, repeats)
    return _cached[key]


def kernel(u, f, time_steps):
    from concourse.bass_utils import run_bass_kernel_spmd

    u = np.asarray(u)
    f = np.asarray(f)
    ts = int(time_steps)
    N = u.shape[0]
    nc = _get_nc(NCORES, ts)
    in_maps = _host_inputs(u, f, NCORES, ts)
    res = run_bass_kernel_spmd(nc, in_maps, list(range(NCORES))).results
    out = np.empty((N, N), dtype=np.float32)
    for c in range(NCORES):
        po = res[c]["pout"]  # [128, 8, 128] -> [128 x, 1024 y]
        out[c * P:(c + 1) * P] = po.transpose(0, 2, 1).reshape(P, N)
    return out
